# revision 23
# baseline (speedup 1.0000x reference)
"""Trainium2 Bass kernel for BottleneckAttention (patch attention).

q patches [160, 5120] from z1_hat (non-overlapping 10x4 unfold),
kv patches [5551, 5120] from z2 (overlapping unfold, Hk=91 x Wk=61),
scores = q @ kv.T / 5120, softmax over kv patches, out = attn @ kv,
folded back to [1, 128, 100, 64].

Sharding: 12 kv h-rows per core (8 x 12 = 96 >= 91); 768 flat positions
= 6 chunks of 128; invalid positions (w >= 61, h >= 91) are masked to
zero on-chip so no host-side softmax corrections are needed.

Fully transposed formulation, all heavy matmuls fp8e4 DoubleRow (two
contraction planes per instruction = 2x bf16 PE throughput):
  phase 1: scoresT[pos, q] implicit conv: psum[pos_chunk, q] accumulates
    z_window.T @ qT over the 40 kernel offsets (i, j); the DoubleRow
    planes are the (2jh, 2jh+1) byte-shifted copies of the core's flat
    z2 slab. Runs in TWO PASSES over all 6 chunks (6 open psum banks):
    pass A (j in {0,1}) needs only the first input DMA, pass B
    (j in {2,3}) the second -- compute starts as soon as possible and
    the input stream (the startup bottleneck) stays fully overlapped.
    Slabs 2,3 are built on-chip by DVE byte-shift copies.
  ACT: e = exp(scores/5120); DVE: f = e*mask - mask -> fp8 eT
    (centered softmax keeps fp8 absolute precision on f = e-1).
  den = ones.T @ f (1-row chain with progressive per-chunk waits,
    interleaved with the r0 chains of the first three phase-2 groups to
    hide the exp/mask pipeline tail at the phase boundary).
  phase 2: out3[c, (i,j), q] += zT_shift.T @ f over position chunks;
    zt holds 8 partition-shifted transposed z copies so every offset is
    a chunk-aligned slice; DoubleRow pairs chunks (t, t+1); M=128
    channels, N=160 queries. Partials ship as fp8 (errors are ~1e-4 of
    the colsum-dominated numerator).

Scheduling notes (measured on hw): input DMAs cost ~115ns per
per-partition packet on each of 16 DMA engines, so the critical first
DMA is kept small and issued pre-block; the PE clock needs ~4us of
continuous work to reach 2.4GHz and decays when idle, so a dummy-tile
warmup is sized to end exactly when the first input lands; the fixed
NEFF overhead (boot + engine teardown) is ~12us and each semaphore adds
~190ns of teardown.

Host: adds the exact colsum term (box-filter sum of z2 windows) to the
numerator, 5551 to the denominator, divides, folds patches back.
"""

import sys

sys.path.insert(0, "/opt/trn_rl_repo")

import numpy as np
import ml_dtypes

import concourse.bass as bass
import concourse.mybir as mybir

DT = mybir.dt
AF = mybir.ActivationFunctionType
ALU = mybir.AluOpType
PM = mybir.MatmulPerfMode.DoubleRow
V = mybir.VecI64Pair

# problem geometry (hardcoded from the reference module)
KC, KH, KW = 128, 10, 4
H, W = 100, 64
NH, NW = H // KH, W // KW          # 10, 16
PQ = NH * NW                       # 160 q patches
D = KC * KH * KW                   # 5120
HK, WK = H - KH + 1, W - KW + 1    # 91, 61
NCORES = 8
HPC = 12                           # kv h-rows per core
PKC = HPC * W                      # 768 flat positions per core
T = PKC // 128                     # 6 position chunks
ZF = 1344                          # z slab flat length (21*64)
ZTC = 10                           # zt chunks
NIJ = KH * KW                      # 40 kernel offsets
SCALE = 1.0 / D
NGRP = 14                          # phase-2 groups of <=3 offsets
SHIFTS = (0, 1, 2, 3, 64, 65, 66, 67)
AW = 2 * ZF + 1280                 # input A1: [z0 | z1 | q pairs 0-3]
A2W = 1920                         # input A2: q pairs 4-9
BW = 3224                          # input B: [qb | fp32 mask] (3224 B)

F8 = ml_dtypes.float8_e4m3

_CACHE = {}


def _build_nc():
    nc = bass.Bass()
    a_d = nc.declare_dram_parameter("a", [KC, AW], DT.float8e4, isOutput=False)
    a2_d = nc.declare_dram_parameter("a2", [KC, A2W], DT.float8e4, isOutput=False)
    b_d = nc.declare_dram_parameter("b", [KC, BW], DT.float8e4, isOutput=False)
    zt_d = nc.declare_dram_parameter(
        "zt", [128, 8, ZTC, KC], DT.float8e4, isOutput=False
    )
    out_d = nc.declare_dram_parameter("out", [KC, NIJ * PQ], DT.float8e4, isOutput=True)
    den_d = nc.declare_dram_parameter("den", [1, PQ], DT.float32, isOutput=True)

    from contextlib import ExitStack

    ctx = ExitStack()
    with ctx:
        a_sb = ctx.enter_context(nc.sbuf_tensor([KC, AW], DT.float8e4))
        a2_sb = ctx.enter_context(nc.sbuf_tensor([KC, A2W], DT.float8e4))
        b_sb = ctx.enter_context(nc.sbuf_tensor([KC, BW], DT.float8e4))
        z23_sb = ctx.enter_context(nc.sbuf_tensor([KC, 2, ZF], DT.float8e4))
        zt_sb = ctx.enter_context(nc.sbuf_tensor([128, 8, ZTC, KC], DT.float8e4))
        e_sb = ctx.enter_context(nc.sbuf_tensor([128, T, PQ], DT.float32))
        eT_sb = ctx.enter_context(nc.sbuf_tensor([128, T, PQ], DT.float8e4))
        o_sb = ctx.enter_context(nc.sbuf_tensor([KC, NIJ * PQ], DT.float8e4))
        den_sb = ctx.enter_context(nc.sbuf_tensor([1, PQ], DT.float32))
        wdum = ctx.enter_context(nc.sbuf_tensor([128, 2, 128], DT.float8e4))
        ones8 = ctx.enter_context(nc.sbuf_tensor([128, 1], DT.float8e4))

        psC = [
            ctx.enter_context(nc.psum_tensor(f"psC_{i}", [128, 512], DT.float32))
            for i in range(T)
        ]
        psD = ctx.enter_context(nc.psum_tensor("psD", [128, 512], DT.float32))
        BANKS = psC + [psD]  # phase-2 7-way rotation

        s_g = ctx.enter_context(nc.semaphore("s_g"))
        s_zA = ctx.enter_context(nc.semaphore("s_zA"))
        s_A2 = ctx.enter_context(nc.semaphore("s_A2"))
        s_qb = ctx.enter_context(nc.semaphore("s_qb"))
        s_zt = ctx.enter_context(nc.semaphore("s_zt"))
        s_pe = ctx.enter_context(nc.semaphore("s_pe"))
        s_e = ctx.enter_context(nc.semaphore("s_e"))
        s_f = ctx.enter_context(nc.semaphore("s_f"))
        s_cpA = ctx.enter_context(nc.semaphore("s_cpA"))
        s_cpV = ctx.enter_context(nc.semaphore("s_cpV"))
        s_o = ctx.enter_context(nc.semaphore("s_o"))

        mk_ap = b_sb[:, 3200:BW].bitcast(DT.float32)  # [128, 6] fp32 mask

        # issue the input DMAs pre-block: the sync engine starts them
        # right after the init barrier, ~0.5us before the block bodies
        nc.sync.dma_start(a_sb[:], a_d[:]).then_inc(s_zA, 16)
        nc.sync.dma_start(a2_sb[:], a2_d[:]).then_inc(s_A2, 16)
        nc.sync.dma_start(b_sb[:], b_d[:]).then_inc(s_qb, 16)
        nc.sync.dma_start(zt_sb[:], zt_d[:]).then_inc(s_zt, 16)

        # 12 groups of 3 offsets + 2 trailing groups of 2: the tail
        # groups are smaller so their output pieces ship sooner
        GROUPS = [list(range(3 * g, 3 * g + 3)) for g in range(12)]
        GROUPS += [[36, 37], [38, 39]]

        def copy_wait(g):
            # (sem, value) signalling group g's psum->sbuf copy completed
            return (s_cpA, g // 2 + 2) if g % 2 == 0 else (s_cpV, g // 2 + 1)

        def z_pair(jh, i_, t):
            # lhsT [128, 2, 128]: byte-shifted slab planes (2jh, 2jh+1);
            # 64-aligned offsets/strides per the dual-fp8 ldweights ISA rules
            a = i_ * W + t * 128
            if jh == 0:
                u = a_sb[:, a : a + 128].unsqueeze(1)
                u.ap = V([[AW, 128], [ZF, 2], [1, 128]])
                return u
            return z23_sb[:, :, a : a + 128]

        def q_pair(pi):
            # rhs [128, 2, 160]: the two q planes of pair pi
            if pi < 4:
                u = a_sb[:, 2 * ZF + pi * 320 : 2 * ZF + pi * 320 + 160].unsqueeze(1)
                u.ap = V([[AW, 128], [PQ, 2], [1, PQ]])
                return u
            if pi < 10:
                u = a2_sb[:, (pi - 4) * 320 : (pi - 4) * 320 + 160].unsqueeze(1)
                u.ap = V([[A2W, 128], [PQ, 2], [1, PQ]])
                return u
            u = b_sb[:, (pi - 10) * 320 : (pi - 10) * 320 + 160].unsqueeze(1)
            u.ap = V([[BW, 128], [PQ, 2], [1, PQ]])
            return u

        def p2_mm(g, r, ij, tp, **kw):
            i_, j_ = ij // KW, ij % KW
            s_idx = (i_ % 2) * 4 + j_
            di = i_ // 2
            return nc.tensor.matmul(
                BANKS[g % 7][0:128, r * PQ : (r + 1) * PQ],
                zt_sb[:, s_idx, 2 * tp + di : 2 * tp + di + 2, :],
                eT_sb[:, 2 * tp : 2 * tp + 2, :],
                start=(tp == 0),
                stop=(tp == 2),
                perf_mode=PM,
                **kw,
            )

        with nc.Block() as block:

            @block.gpsimd
            def _(g):
                g.memset(wdum[:], 0.0)
                g.memset(ones8[:], 1.0).then_inc(s_g, 1)

            @block.sync
            def _(sync):
                sync.wait_ge(s_cpA, 1)
                sync.dma_start(den_d[:], den_sb[:]).then_inc(s_o, 16)
                PIECES = [(0, 2880, 4, 3), (2880, 5760, 7, 6),
                          (5760, 6080, 8, 6), (6080, 6400, 8, 7)]
                for c0, c1, na, nv in PIECES:
                    sync.wait_ge(s_cpA, na)
                    sync.wait_ge(s_cpV, nv)
                    sync.dma_start(out_d[:, c0:c1], o_sb[:, c0:c1]).then_inc(s_o, 16)
                sync.wait_ge(s_o, 80)

            @block.tensor
            def _(pe):
                # HAM warmup sized to end as the first input DMA completes
                # (~10.7us): the clock ramp then finishes exactly when real
                # work can start
                pe.wait_ge(s_g, 1)
                for w_ in range(32):
                    nc.tensor.matmul(
                        psD[0:128, 0:128],
                        wdum[:, :, :],
                        wdum[:, :, :],
                        start=(w_ == 0),
                        stop=(w_ == 31),
                        perf_mode=PM,
                    )
                # phase 1, pass A (j in {0,1}): open all 6 chunk accumulators
                pe.wait_ge(s_zA, 16)
                for t in range(T):
                    for pi in range(4):
                        nc.tensor.matmul(
                            psC[t][0:128, 0:PQ],
                            z_pair(0, pi, t),
                            q_pair(pi),
                            start=(pi == 0),
                            stop=False,
                            perf_mode=PM,
                        )
                pe.wait_ge(s_A2, 16)
                for t in range(T):
                    for pi in range(4, 10):
                        nc.tensor.matmul(
                            psC[t][0:128, 0:PQ],
                            z_pair(0, pi, t),
                            q_pair(pi),
                            start=False,
                            stop=False,
                            perf_mode=PM,
                        )
                # pass B (j in {2,3}): close them
                pe.wait_ge(s_qb, 16)
                pe.wait_ge(s_f, 2)  # z23 slab copies done
                for t in range(T):
                    for pi in range(10, 20):
                        mm = nc.tensor.matmul(
                            psC[t][0:128, 0:PQ],
                            z_pair(1, pi - 10, t),
                            q_pair(pi),
                            start=False,
                            stop=(pi == 19),
                            perf_mode=PM,
                        )
                    mm.then_inc(s_pe, 1)  # 1..6

                # den chain (plain fp8, psD) interleaved with the first
                # phase-2 chain under progressive f waits
                def den_mm(t):
                    return nc.tensor.matmul(
                        psD[0:1, 0:PQ],
                        ones8[:, 0:1],
                        eT_sb[:, t, :],
                        start=(t == 0),
                        stop=(t == T - 1),
                    )

                # interleave den (psD) and the r0 chains of groups 0..2
                # (banks psC_0..2) under progressive f waits: ~15 matmuls of
                # fill that hide the exp/mask pipeline tail of chunks 4,5
                pe.wait_ge(s_g, 1)
                pe.wait_ge(s_f, 3)
                den_mm(0)
                pe.wait_ge(s_f, 4)
                den_mm(1)
                pe.wait_ge(s_zt, 16)
                pe.wait_ge(s_e, 3)  # psC_2 drained before g2's r0 opens
                for g in range(3):
                    p2_mm(g, 0, GROUPS[g][0], 0)
                pe.wait_ge(s_f, 5)
                den_mm(2)
                pe.wait_ge(s_f, 6)
                den_mm(3)
                for g in range(3):
                    p2_mm(g, 0, GROUPS[g][0], 1)
                pe.wait_ge(s_f, 7)
                den_mm(4)
                pe.wait_ge(s_f, 8)
                den_mm(5).then_inc(s_pe, 1)  # 7
                for g in range(3):
                    p2_mm(g, 0, GROUPS[g][0], 2)
                for g in range(3):
                    for r in (1, 2):
                        for tp in range(3):
                            mm = p2_mm(g, r, GROUPS[g][r], tp)
                    mm.then_inc(s_pe, 1)  # 8 + g

                # phase-2 groups 3..13
                for g in range(3, NGRP):
                    if g <= 5:
                        pe.wait_ge(s_e, g + 1)  # psC[g] drained by exp g
                    elif g == 6:
                        pe.wait_ge(s_cpA, 1)  # psD drained by den copy
                    else:
                        sem, val = copy_wait(g - 7)
                        pe.wait_ge(sem, val)
                    for r, ij in enumerate(GROUPS[g]):
                        for tp in range(3):
                            mm = p2_mm(g, r, ij, tp)
                    mm.then_inc(s_pe, 1)  # 8 + g

            @block.scalar
            def _(act):
                for t in range(T):
                    act.wait_ge(s_pe, t + 1)
                    nc.scalar.activation(
                        e_sb[:, t, :], psC[t][0:128, 0:PQ], AF.Exp, scale=SCALE
                    ).then_inc(s_e, 1)
                act.wait_ge(s_pe, 7)
                nc.scalar.activation(
                    den_sb[:], psD[0:1, 0:PQ], AF.Copy
                ).then_inc(s_cpA, 1)
                for g in range(0, NGRP, 2):
                    act.wait_ge(s_pe, 8 + g)
                    c0 = PQ * GROUPS[g][0]
                    ncol = len(GROUPS[g]) * PQ
                    nc.scalar.activation(
                        o_sb[:, c0 : c0 + ncol],
                        BANKS[g % 7][0:128, 0:ncol],
                        AF.Copy,
                    ).then_inc(s_cpA, 1)

            @block.vector
            def _(dve):
                # build byte-shifted slabs 2,3 from the landed z slabs 0,1
                # (the 2 trailing bytes read stray qa data -- they only feed
                # masked w>=62 positions)
                dve.wait_ge(s_zA, 16)
                nc.vector.tensor_copy(
                    z23_sb[:, 0, :], a_sb[:, 2 : 2 + ZF]
                ).then_inc(s_f, 1)
                nc.vector.tensor_copy(
                    z23_sb[:, 1, :], a_sb[:, ZF + 2 : 2 * ZF + 2]
                ).then_inc(s_f, 1)
                for t in range(T):
                    if t == 0:
                        dve.wait_ge(s_qb, 16)
                    dve.wait_ge(s_e, t + 1)
                    nc.vector.tensor_scalar(
                        eT_sb[:, t, :],
                        e_sb[:, t, :],
                        mk_ap[:, t : t + 1],
                        mk_ap[:, t : t + 1],
                        ALU.mult,
                        ALU.subtract,
                    ).then_inc(s_f, 1)
                for g in range(1, NGRP, 2):
                    dve.wait_ge(s_pe, 8 + g)
                    c0 = PQ * GROUPS[g][0]
                    ncol = len(GROUPS[g]) * PQ
                    nc.vector.tensor_copy(
                        o_sb[:, c0 : c0 + ncol],
                        BANKS[g % 7][0:128, 0:ncol],
                    ).then_inc(s_cpV, 1)

    return nc


def _host_prep(z1_hat, z2):
    z1 = np.asarray(z1_hat, dtype=np.float32)[0]  # [128, 100, 64]
    z2a = np.asarray(z2, dtype=np.float32)[0]

    # q patches -> paired rhs layout [128, 20, 2, 160] fp8, flattened
    q = z1.reshape(KC, NH, KH, NW, KW).transpose(1, 3, 0, 2, 4).reshape(PQ, D)
    qT3 = q.reshape(PQ, KC, KH * KW).transpose(1, 2, 0)  # [128, 40, 160]
    qp = (
        qT3.reshape(KC, NH, 2, 2, PQ)
        .transpose(0, 2, 1, 3, 4)
        .reshape(KC, 6400)
        .astype(F8)
    )
    qpb = np.ascontiguousarray(qp).view(np.uint8)  # [128, 6400]

    # padded z2 (rows 100..111 zero), fp8, flattened
    z_pad = np.zeros((KC, 112, W), dtype=np.float32)
    z_pad[:, :H] = z2a
    z8 = z_pad.astype(F8).reshape(KC, 112 * W)
    z8u = z8.view(np.uint8)
    z8T = np.ascontiguousarray(z8.T)  # [7168, 128]

    in_maps = []
    for core in range(NCORES):
        base = HPC * core * W
        a = np.ascontiguousarray(
            np.concatenate(
                [z8u[:, base : base + ZF], z8u[:, base + 1 : base + 1 + ZF],
                 qpb[:, 0:1280]],
                axis=1,
            )
        ).view(F8)
        a2 = np.ascontiguousarray(qpb[:, 1280:3200]).view(F8)
        zt = np.ascontiguousarray(
            np.stack(
                [
                    z8T[base + s : base + s + ZTC * 128]
                    .reshape(ZTC, 128, KC)
                    .transpose(1, 0, 2)
                    for s in SHIFTS
                ],
                axis=1,
            )
        )
        pos = np.arange(PKC)
        valid = (pos % W < WK) & (HPC * core + pos // W < HK)
        mk = np.ascontiguousarray(valid.astype(np.float32).reshape(T, 128).T)
        b = np.ascontiguousarray(
            np.concatenate([qpb[:, 3200:6400], mk.view(np.uint8)], axis=1)
        ).view(F8)
        in_maps.append({"a": a, "a2": a2, "b": b, "zt": zt})

    # exact colsum term: colsum[c, i, j] = sum_{h<91, w<61} z2[c, h+i, w+j]
    ii = np.zeros((KC, H + 1, W + 1), dtype=np.float64)
    ii[:, 1:, 1:] = np.cumsum(np.cumsum(z2a, axis=1), axis=2)
    colsum = np.empty((KC, KH, KW), dtype=np.float64)
    for i in range(KH):
        for j in range(KW):
            colsum[:, i, j] = (
                ii[:, i + HK, j + WK] - ii[:, i, j + WK] - ii[:, i + HK, j] + ii[:, i, j]
            )
    return in_maps, colsum.reshape(KC, NIJ)


def kernel(z1_hat, z2):
    from concourse.bass_utils import run_bass_kernel_spmd

    in_maps, colsum = _host_prep(z1_hat, z2)
    if "nc" not in _CACHE:
        _CACHE["nc"] = _build_nc()
    nc = _CACHE["nc"]
    res = run_bass_kernel_spmd(nc, in_maps, list(range(NCORES)))
    num = np.broadcast_to(colsum[:, :, None], (KC, NIJ, PQ)).astype(np.float64).copy()
    den = np.full((PQ,), float(HK * WK), dtype=np.float64)
    for r in res.results:
        num += r["out"].astype(np.float64).reshape(KC, NIJ, PQ)
        den += r["den"].astype(np.float64)[0]
    out4 = num / den[None, None, :]
    # fold back: [c, (kh,kw), (nh,nw)] -> [1, 128, 100, 64]
    out4 = out4.reshape(KC, KH, KW, NH, NW).transpose(0, 3, 1, 4, 2)
    return np.ascontiguousarray(out4.reshape(1, KC, H, W).astype(np.float32))
